# revision 1
# baseline (speedup 1.0000x reference)
"""Trainium2 Bass kernel for MultiHeadLinearAttention.

Problem: B=4, S=2048, D=1024, H=16 heads of hd=64.
  q,k,v = x@W + b ; q,k = elu(q|k)+1
  recurrent scan: s += k_t v_t^T ; z += k_t ; out_t = (q_t s)/(q_t z + 1e-6)
  y = out @ Wo + bo

Strategy (8 NeuronCores):
  core c -> batch b = c//2, heads hh = (c%2)*8 .. +8  (column-sliced Wq/Wk/Wv,
  row-sliced Wo; the two cores of a batch produce partial output-projection
  sums that the host adds together, plus bo).

  Linear attention is computed in chunked (block-parallel) form with chunk
  C=128: per chunk and head,
    AT[j,i] = sum_d k[j,d] q[i,d]          (j,i in chunk; masked to j<=i)
    acc[i,0:64]  = sum_j AT_m[j,i] v[j,:] + q_i @ S_pre
    acc[i,64]    = sum_j AT_m[j,i] * 1    + q_i @ z_pre    ([V|1] interleave)
    out_i = acc[i,0:64] / (acc[i,64] + 1e-6)
    [S|z] += K_c^T [V_c|1]
  elu(x)+1 == min(exp(x),1) + max(x,0).

  The host feeds x[b].T so projections contract along the partition dim and
  produce Q^T,K^T,V^T directly; K/V natural layouts come from PE transposes.
"""

import os
import sys

import numpy as np
import ml_dtypes

sys.path.insert(0, "/opt/trn_rl_repo")

B, S, D = 4, 2048, 1024
H, HD = 16, 64
HPC = 8           # heads per core
MC = HPC * HD     # 512 head-dims per core
C = 128           # attention chunk
SB = 512          # s-block
NBLK = S // SB    # 4
NST = SB // C     # s-tiles (=chunks) per block

# matmul operand dtype: "bfloat16" | "float32r" | "float32"
DT_NAME = os.environ.get("BASS_LINATTN_DT", "bfloat16")

_built = {}


def _np_dt(name):
    return {"bfloat16": ml_dtypes.bfloat16,
            "float32r": np.float32,
            "float32": np.float32}[name]


def _build(dt_name):
    import concourse.bass as bass
    import concourse.mybir as mybir
    from concourse import bacc
    from concourse.tile import TileContext

    DT = getattr(mybir.dt, dt_name)
    F32 = mybir.dt.float32
    AF = mybir.ActivationFunctionType
    ALU = mybir.AluOpType

    nc = bacc.Bacc("TRN2", target_bir_lowering=False, debug=False)

    xt = nc.dram_tensor("xt", (D, S), DT, kind="ExternalInput")
    wq = nc.dram_tensor("wq", (D, MC), DT, kind="ExternalInput")
    wk = nc.dram_tensor("wk", (D, MC), DT, kind="ExternalInput")
    wv = nc.dram_tensor("wv", (D, MC), DT, kind="ExternalInput")
    wo = nc.dram_tensor("wo", (MC, D), DT, kind="ExternalInput")
    bqkv = nc.dram_tensor("bqkv", (128, 12), F32, kind="ExternalInput")
    bvb = nc.dram_tensor("bvb", (128, MC), F32, kind="ExternalInput")
    msk = nc.dram_tensor("msk", (128, 256), F32, kind="ExternalInput")
    idn = nc.dram_tensor("idn", (128, 128), DT, kind="ExternalInput")
    out = nc.dram_tensor("out", (S, D), F32, kind="ExternalOutput")

    KT = D // 128          # 8 k-tiles of the contraction dim
    DT_TILES = MC // 128   # 4 tiles of per-core head dims

    with TileContext(nc) as tc:
        with (
            tc.tile_pool(name="consts", bufs=1) as consts,
            tc.tile_pool(name="xt_pool", bufs=2) as xt_pool,
            tc.tile_pool(name="qkvt", bufs=2) as qkvt,
            tc.tile_pool(name="nat", bufs=2) as nat,
            tc.tile_pool(name="attn_pool", bufs=8) as attn_pool,
            tc.tile_pool(name="attnT_pool", bufs=8) as attnT_pool,
            tc.tile_pool(name="state_pool", bufs=1) as state_pool,
            tc.tile_pool(name="small", bufs=8) as small,
            tc.tile_pool(name="evac", bufs=4) as evac,
            tc.tile_pool(name="psA", bufs=2, space="PSUM") as psA,
            tc.tile_pool(name="psB", bufs=2, space="PSUM") as psB,
        ):
            # ---- constants -------------------------------------------------
            wq_sb = consts.tile([128, KT, MC], DT)
            wk_sb = consts.tile([128, KT, MC], DT)
            wv_sb = consts.tile([128, KT, MC], DT)
            for t, w in ((wq_sb, wq), (wk_sb, wk), (wv_sb, wv)):
                nc.sync.dma_start(t, w.rearrange("(kt p) n -> p kt n", p=128))
            wo_sb = consts.tile([128, DT_TILES, D], DT)
            nc.sync.dma_start(wo_sb, wo.rearrange("(mt p) n -> p mt n", p=128))
            bias_sb = consts.tile([128, 12], F32)
            nc.sync.dma_start(bias_sb, bqkv[:, :])
            bvb_sb = consts.tile([128, MC], F32)
            nc.sync.dma_start(bvb_sb, bvb[:, :])
            mask_sb = consts.tile([128, 256], F32)   # [triu | triu]
            nc.sync.dma_start(mask_sb, msk[:, :])
            ident = consts.tile([128, 128], DT)
            nc.sync.dma_start(ident, idn[:, :])

            # ---- recurrent state [S|z], head pairs, block-diagonal --------
            state_b = state_pool.tile([128, HPC // 2, 130], DT)
            nc.vector.memset(state_b, 0.0)

            out_r = out.rearrange("(st p) n -> st p n", p=128)

            # per-block tiles, lazily created by the P stage
            T = {}

            def stage_p_steps(blk):
                """Projection-phase emission steps for block blk."""
                ssl = slice(blk * SB, (blk + 1) * SB)

                def dma_step():
                    xt_t = xt_pool.tile([128, KT, SB], DT, tag="xt",
                                        name=f"xt_{blk}")
                    T["xt", blk] = xt_t
                    nc.sync.dma_start(
                        xt_t, xt.rearrange("(kt p) s -> p kt s", p=128)[:, :, ssl]
                    )
                    T["qt", blk] = qkvt.tile([128, DT_TILES, SB], DT, tag="qt",
                                             name=f"qt_{blk}")
                    T["kt", blk] = qkvt.tile([128, DT_TILES, SB], DT, tag="kt",
                                             name=f"kt_{blk}")
                    T["knat", blk] = nat.tile([128, NST, MC], DT, tag="knat",
                                              name=f"knat_{blk}")
                    vnat = nat.tile([128, NST, HPC * 65], DT, tag="vnat",
                                    name=f"vnat_{blk}")
                    T["vnat", blk] = vnat
                    nc.vector.memset(
                        vnat.rearrange("p st (h e) -> p st h e", e=65)
                            [:, :, :, 64:65], 1.0
                    )
                yield dma_step

                for di in range(DT_TILES):
                    def qk_step(di=di):
                        dsl = slice(di * 128, (di + 1) * 128)
                        xt_t = T["xt", blk]
                        for pname, w_sb, bcol, dkey in (
                            ("q", wq_sb, di, "qt"),
                            ("k", wk_sb, 4 + di, "kt"),
                        ):
                            dst = T[dkey, blk]
                            ps = psA.tile([128, SB], F32, tag="proj", bufs=3,
                                          name=f"ps_{pname}{di}_{blk}")
                            for k in range(KT):
                                nc.tensor.matmul(
                                    ps, w_sb[:, k, dsl], xt_t[:, k],
                                    start=(k == 0), stop=(k == KT - 1),
                                )
                            bias_ap = bias_sb[:, bcol:bcol + 1]
                            # elu(u)+1 = min(exp(u),1) + max(u,0), u = ps+bias
                            e = evac.tile([128, SB], DT, tag="e",
                                          name=f"e_{pname}{di}_{blk}")
                            nc.scalar.activation(e, ps, AF.Exp, bias=bias_ap)
                            r = evac.tile([128, SB], DT, tag="r",
                                          name=f"r_{pname}{di}_{blk}")
                            nc.scalar.activation(r, ps, AF.Relu, bias=bias_ap)
                            nc.vector.scalar_tensor_tensor(
                                out=dst[:, di], in0=e, scalar=1.0, in1=r,
                                op0=ALU.min, op1=ALU.add,
                            )
                        if dkey == "kt":
                            pass
                    yield qk_step

                # natural-layout K via PE transpose of elu'd Kt
                for di in range(DT_TILES):
                    def ktr_step(di=di):
                        kt_t = T["kt", blk]
                        knat = T["knat", blk]
                        for st in range(NST):
                            csl = slice(st * 128, (st + 1) * 128)
                            tr = psB.tile([128, 128], DT, tag="at", bufs=2,
                                          name=f"trk{di}_{st}_{blk}")
                            nc.tensor.transpose(tr, kt_t[:, di, csl], ident)
                            nc.any.tensor_copy(
                                knat[:, st, di * 128:(di + 1) * 128], tr
                            )
                    yield ktr_step

                # natural-layout V via direct (natural-out) projection
                for st in range(NST):
                    def vnat_step(st=st):
                        xt_t = T["xt", blk]
                        vnat = T["vnat", blk]
                        stsl = slice(st * 128, (st + 1) * 128)
                        ps = psA.tile([128, MC], F32, tag="proj", bufs=3,
                                      name=f"ps_vn{st}_{blk}")
                        for k in range(KT):
                            nc.tensor.matmul(
                                ps, T["xt", blk][:, k, stsl], wv_sb[:, k],
                                start=(k == 0), stop=(k == KT - 1),
                            )
                        nc.vector.tensor_add(
                            vnat.rearrange("p st (h e) -> p st h e", e=65)
                                [:, st, :, 0:64],
                            ps.rearrange("p (h e) -> p h e", e=64),
                            bvb_sb.rearrange("p (h e) -> p h e", e=64),
                        )
                    yield vnat_step

            def stage_a_steps(blk):
                """Attention + output-projection emission steps for block blk."""
                def alloc_step():
                    T["attn", blk] = [
                        attn_pool.tile([128, MC], DT, tag="attn",
                                       name=f"attn{st}_{blk}")
                        for st in range(NST)
                    ]
                    T["attnT", blk] = [
                        attnT_pool.tile([128, SB], DT, tag="attnT",
                                        name=f"attnT{p}_{blk}")
                        for p in range(DT_TILES)
                    ]
                yield alloc_step

                for cc in range(NST):
                    for hp in range(HPC // 2):
                        def pair_step(cc=cc, hp=hp):
                            csl = slice(cc * 128, (cc + 1) * 128)
                            qt_t, kt_t = T["qt", blk], T["kt", blk]
                            knat, vnat = T["knat", blk], T["vnat", blk]
                            # per-head AT psum tiles: the two matmuls run
                            # concurrently (different PE row groups), so they
                            # must write DIFFERENT psum banks
                            at_m = small.tile([128, 256], DT, tag="atm",
                                              name=f"atm{hp}_{cc}_{blk}")
                            for o in range(2):
                                pr = slice(o * 64, o * 64 + 64)
                                at_ps = psB.tile([128, 128], F32, tag="at", bufs=2,
                                                 name=f"at{hp}{o}_{cc}_{blk}")
                                nc.tensor.matmul(
                                    at_ps, kt_t[pr, hp, csl], qt_t[pr, hp, csl],
                                    start=True, stop=True,
                                )
                                nc.vector.tensor_mul(
                                    at_m[:, o * 128:(o + 1) * 128], at_ps,
                                    mask_sb[:, 0:128],
                                )
                            # acc = [num_e | den_e | num_o | den_o]
                            # NOTE: the inter matmul opens the accumulation
                            # group (start=True zeroes the whole PSUM bank;
                            # sub-bank disjoint start=True writes would
                            # clobber each other).
                            acc = psB.tile([128, 130], F32, tag="acc",
                                           name=f"acc{hp}_{cc}_{blk}")
                            nc.tensor.matmul(
                                acc, qt_t[:, hp, csl], state_b[:, hp],
                                start=True, stop=False, skip_group_check=True,
                            )
                            for o in range(2):
                                h = 2 * hp + o
                                nc.tensor.matmul(
                                    acc[:, o * 65:o * 65 + 65],
                                    at_m[:, o * 128:(o + 1) * 128],
                                    vnat[:, cc, h * 65:(h + 1) * 65],
                                    start=False, stop=(o == 1),
                                    skip_group_check=True,
                                )
                            # state += K_c^T [V|1] (pair; off-diag blocks unused)
                            stp = psB.tile([128, 130], F32, tag="state", bufs=1,
                                           name=f"stp{hp}_{cc}_{blk}")
                            nc.tensor.matmul(
                                stp, knat[:, cc, hp * 128:(hp + 1) * 128],
                                vnat[:, cc, hp * 130:(hp + 1) * 130],
                                start=True, stop=True,
                            )
                            # paired reciprocal of the two den columns
                            rec = small.tile([128, 2], F32, tag="rec",
                                             name=f"rec{hp}_{cc}_{blk}")
                            nc.vector.reciprocal(rec, acc[:, 64:130:65])
                            for o in range(2):
                                pr = slice(o * 64, o * 64 + 64)
                                osl = slice(o * 65, o * 65 + 65)
                                nc.vector.tensor_add(
                                    state_b[pr, hp, osl], state_b[pr, hp, osl],
                                    stp[pr, osl],
                                )
                                h = 2 * hp + o
                                nc.vector.tensor_scalar_mul(
                                    T["attn", blk][cc][:, h * 64:(h + 1) * 64],
                                    acc[:, o * 65:o * 65 + 64], rec[:, o:o + 1],
                                )
                        yield pair_step

                    def attnT_step(cc=cc):
                        csl = slice(cc * 128, (cc + 1) * 128)
                        for p in range(DT_TILES):
                            trA = psB.tile([128, 128], DT, tag="at", bufs=2,
                                           name=f"trA{p}_{cc}_{blk}")
                            nc.tensor.transpose(
                                trA, T["attn", blk][cc][:, p * 128:(p + 1) * 128],
                                ident,
                            )
                            nc.any.tensor_copy(T["attnT", blk][p][:, csl], trA)
                    yield attnT_step

                for st in range(NST):
                    for nb in range(D // 512):
                        def oproj_step(st=st, nb=nb):
                            csl = slice(st * 128, (st + 1) * 128)
                            nsl = slice(nb * 512, (nb + 1) * 512)
                            ops = psA.tile([128, 512], F32, tag="proj", bufs=3,
                                           name=f"ops{st}_{nb}_{blk}")
                            for p in range(DT_TILES):
                                nc.tensor.matmul(
                                    ops, T["attnT", blk][p][:, csl],
                                    wo_sb[:, p, nsl],
                                    start=(p == 0), stop=(p == DT_TILES - 1),
                                )
                            ob = evac.tile([128, 512], F32, tag="ob",
                                           name=f"ob{st}_{nb}_{blk}")
                            nc.vector.tensor_copy(ob, ops)
                            nc.sync.dma_start(out_r[blk * NST + st, :, nsl], ob)
                        yield oproj_step

            # ---- software-pipelined emission ------------------------------
            for step in stage_p_steps(0):
                step()
            for blk in range(NBLK):
                a_steps = list(stage_a_steps(blk))
                p_steps = list(stage_p_steps(blk + 1)) if blk + 1 < NBLK else []
                # interleave: spread p_steps evenly through a_steps
                na, npp = len(a_steps), len(p_steps)
                pi = 0
                for i, astep in enumerate(a_steps):
                    astep()
                    while pi < npp and (i + 1) * npp >= (pi + 1) * na:
                        p_steps[pi]()
                        pi += 1
                while pi < npp:
                    p_steps[pi]()
                    pi += 1

    nc.compile()
    return nc


def _prep_inputs(x, Wq, bq, Wk, bk, Wv, bv, Wo, bo, np_dt):
    f32 = np.float32
    tri = np.triu(np.ones((128, 128), f32))  # mask[j,i] = 1 iff j <= i
    mask_tri = np.concatenate([tri, tri], axis=1)  # paired heads
    ident = np.eye(128, dtype=np_dt)
    in_maps = []
    for c in range(8):
        b, hh = divmod(c, 2)
        cols = slice(hh * MC, (hh + 1) * MC)
        bqkv = np.concatenate(
            [np.asarray(v[cols], f32).reshape(4, 128).T for v in (bq, bk, bv)],
            axis=1,
        ).astype(f32)
        in_maps.append({
            "xt": np.ascontiguousarray(np.asarray(x[b], f32).T).astype(np_dt),
            "wq": np.ascontiguousarray(np.asarray(Wq, f32)[:, cols]).astype(np_dt),
            "wk": np.ascontiguousarray(np.asarray(Wk, f32)[:, cols]).astype(np_dt),
            "wv": np.ascontiguousarray(np.asarray(Wv, f32)[:, cols]).astype(np_dt),
            "wo": np.ascontiguousarray(np.asarray(Wo, f32)[cols, :]).astype(np_dt),
            "bqkv": np.ascontiguousarray(bqkv),
            "bvb": np.ascontiguousarray(
                np.tile(np.asarray(bv, f32)[cols][None, :], (128, 1))
            ),
            "msk": mask_tri,
            "idn": ident,
        })
    return in_maps


def run(inputs, trace=False):
    """Run the kernel; returns (full_output, BassKernelResults)."""
    from concourse.bass_utils import run_bass_kernel_spmd

    dt_name = DT_NAME
    if dt_name not in _built:
        _built[dt_name] = _build(dt_name)
    nc = _built[dt_name]

    x = np.asarray(inputs["x"], np.float32)
    bo = np.asarray(inputs["bo"], np.float32)
    in_maps = _prep_inputs(
        x, inputs["Wq"], inputs["bq"], inputs["Wk"], inputs["bk"],
        inputs["Wv"], inputs["bv"], inputs["Wo"], bo, _np_dt(dt_name),
    )
    res = run_bass_kernel_spmd(
        nc, in_maps, core_ids=list(range(8)), trace=trace,
        trace_cores=list(range(8)) if trace else None,
    )
    outs = [np.asarray(r["out"], np.float32) for r in res.results]
    full = np.empty((B, S, D), np.float32)
    for b in range(B):
        full[b] = outs[2 * b] + outs[2 * b + 1] + bo[None, :]
    return full, res


def kernel(**inputs):
    full, _ = run(inputs, trace=False)
    return full



# revision 26
# speedup vs baseline: 1.3394x; 1.3394x over previous
"""Trainium2 Bass kernel for MultiHeadLinearAttention.

Problem: B=4, S=2048, D=1024, H=16 heads of hd=64.
  q,k,v = x@W + b ; q,k = elu(q|k)+1
  recurrent scan: s += k_t v_t^T ; z += k_t ; out_t = (q_t s)/(q_t z + 1e-6)
  y = out @ Wo + bo

Strategy (8 NeuronCores):
  core c -> batch b = c//2, heads hh = (c%2)*8 .. +8  (column-sliced Wq/Wk/Wv,
  row-sliced Wo; the two cores of a batch produce partial output-projection
  sums that the host adds together, plus bo).

  Linear attention in chunked form, chunk C=128, software-pipelined at chunk
  granularity:
    AT[j,i] = sum_d k[j,d] q[i,d]   (masked j<=i, 4 heads per PSUM bank,
                                     ONE mask-multiply per bank)
    acc[i, h*65:+65] = [ q_i @ [S|z]_pre  +  sum_j AT_m[j,i] [v_j|1] ]
    out_i = acc[..0:64] * recip(acc[..64])    (recip on DVE, scale on Scalar)
    [S|z] += K_c^T [V_c|1]   accumulated IN PSUM across all chunks; one
                             f32->bf16 copy per chunk materializes the
                             state snapshot used by the next chunk's inter.
  elu(x)+1 == min(exp(x),1) + max(x,0)  (2 scalar acts + 1 gpsimd STT).
  V-bias is a rank-1 ones-row matmul folded into the V projection.
  Output projection consumes PE-transposed attn tiles; y is written bf16.
"""

import os
import sys

import numpy as np
import ml_dtypes

sys.path.insert(0, "/opt/trn_rl_repo")

B, S, D = 4, 2048, 1024
H, HD = 16, 64
HPC = 8           # heads per core
MC = HPC * HD     # 512 head-dims per core
C = 128           # attention chunk
SB = 512          # s-block (projection granularity)
NBLK = S // SB    # 4
NST = SB // C     # chunks per block
NCH = S // C      # 16 global chunks

# matmul operand dtype: "bfloat16" | "float32r" | "float32"
DT_NAME = os.environ.get("BASS_LINATTN_DT", "bfloat16")
STT_ENGINE = os.environ.get("BASS_LINATTN_STT", "vector")

_built = {}


def _np_dt(name):
    return {"bfloat16": ml_dtypes.bfloat16,
            "float32r": np.float32,
            "float32": np.float32}[name]


def _build(dt_name):
    import concourse.bass as bass
    import concourse.mybir as mybir
    from concourse import bacc
    from concourse.tile import TileContext

    DT = getattr(mybir.dt, dt_name)
    BF16 = mybir.dt.bfloat16
    F32 = mybir.dt.float32
    AF = mybir.ActivationFunctionType
    ALU = mybir.AluOpType

    nc = bacc.Bacc("TRN2", target_bir_lowering=False, debug=False)

    xt = nc.dram_tensor("xt", (D, S), DT, kind="ExternalInput")
    wq = nc.dram_tensor("wq", (D, MC), DT, kind="ExternalInput")
    wk = nc.dram_tensor("wk", (D, MC), DT, kind="ExternalInput")
    wv = nc.dram_tensor("wv", (D, MC), DT, kind="ExternalInput")
    wo = nc.dram_tensor("wo", (MC, D), DT, kind="ExternalInput")
    # cpack = [ mask (512 cols: 4x triu) | biases (12 cols: q di, k di, v di) ]
    cpack = nc.dram_tensor("cpack", (128, 524), F32, kind="ExternalInput")
    bvrow = nc.dram_tensor("bvrow", (32, MC), DT, kind="ExternalInput")
    idn = nc.dram_tensor("idn", (128, 128), DT, kind="ExternalInput")
    out = nc.dram_tensor("out", (S, D), BF16, kind="ExternalOutput")

    KT = D // 128          # 8 k-tiles of the contraction dim
    DT_TILES = MC // 128   # 4 tiles of per-core head dims

    stt_eng = "gpsimd" if STT_ENGINE == "gpsimd" else "vector"

    with TileContext(nc) as tc:
        with (
            tc.tile_pool(name="consts", bufs=1) as consts,
            tc.tile_pool(name="xt_pool", bufs=2) as xt_pool,
            tc.tile_pool(name="qkvt", bufs=2) as qkvt,
            tc.tile_pool(name="nat", bufs=2) as nat,
            tc.tile_pool(name="attn_pool", bufs=2) as attn_pool,
            tc.tile_pool(name="attnT_pool", bufs=2) as attnT_pool,
            tc.tile_pool(name="small", bufs=2) as small,
            tc.tile_pool(name="evac", bufs=2) as evac,
            tc.tile_pool(name="ob_pool", bufs=2) as ob_pool,
            tc.tile_pool(name="psA", bufs=3, space="PSUM") as psA,
            tc.tile_pool(name="psB", bufs=2, space="PSUM") as psB,
        ):
            # ---- constant tiles -------------------------------------------
            cpk = consts.tile([128, 524], F32)
            ident = consts.tile([128, 128], DT)
            bvrow_sb = consts.tile([32, MC], DT)
            ones32 = consts.tile([32, 128], DT)
            onesb = consts.tile([128, SB], DT)
            wq_sb = consts.tile([128, KT, MC], DT)
            wk_sb = consts.tile([128, KT, MC], DT)
            wv_sb = consts.tile([128, KT, MC], DT)
            wo_sb = consts.tile([128, DT_TILES, D], DT)

            mask_ap = cpk[:, 0:512]

            # ---- persistent [S|z] state: f32 accumulation in PSUM ---------
            # full-bank stride [128, 512]; pair hp's [S|z] lives at cols
            # hp*128 .. hp*128+65 (head 2hp on partitions 0:64, 2hp+1 on 64:128)
            state_ps = psB.tile([128, 4 * 128], F32, tag="state", bufs=1)

            # ---- DMAs: startup-critical first -----------------------------
            nc.sync.dma_start(cpk, cpack[:, :])
            nc.sync.dma_start(ident, idn[:, :])
            nc.sync.dma_start(bvrow_sb, bvrow[:, :])
            nc.gpsimd.memset(ones32, 0.0)
            nc.gpsimd.memset(ones32[0:1], 1.0)
            nc.gpsimd.memset(onesb, 1.0)
            # zero the state bank; every state update accumulates with
            # start=False (overwrite-of-0 and add-to-0 are equivalent)
            nc.vector.memset(state_ps, 0.0)
            # block-diagonal bf16 snapshot of [S|z] per pair: rows 0:64 hold
            # head 2hp at cols 0:65, rows 64:128 hold head 2hp+1 at 65:130.
            # Off-diagonal zeros are set once and never rewritten.
            ssb = consts.tile([128, HPC // 2, 130], DT)
            nc.vector.memset(ssb, 0.0)

            xt_r = xt.rearrange("(kt p) s -> p kt s", p=128)
            wq_r = wq.rearrange("(kt p) n -> p kt n", p=128)
            wk_r = wk.rearrange("(kt p) n -> p kt n", p=128)
            wv_r = wv.rearrange("(kt p) n -> p kt n", p=128)
            out_r = out.rearrange("(st p) n -> st p n", p=128)

            T = {}

            def alloc_block(blk):
                T["xt", blk] = xt_pool.tile([128, KT, SB], DT, tag="xt",
                                            name=f"xt_{blk}")
                T["qt", blk] = qkvt.tile([128, DT_TILES, SB], DT, tag="qt",
                                         name=f"qt_{blk}")
                T["kt", blk] = qkvt.tile([128, DT_TILES, SB], DT, tag="kt",
                                         name=f"kt_{blk}")
                T["knat", blk] = nat.tile([128, NST, MC], DT, tag="knat",
                                          name=f"knat_{blk}")
                vnat = nat.tile([128, NST, HPC * 65], DT, tag="vnat",
                                name=f"vnat_{blk}")
                T["vnat", blk] = vnat
                nc.gpsimd.memset(
                    vnat.rearrange("p st (h e) -> p st h e", e=65)
                        [:, :, :, 64:65], 1.0
                )
                T["attn", blk] = attn_pool.tile([128, NST, MC], DT, tag="attn",
                                                name=f"attn_{blk}")
                T["attnT", blk] = attnT_pool.tile([128, DT_TILES, SB], DT,
                                                  tag="attnT",
                                                  name=f"attnT_{blk}")

            # block 0: per-ktile weight/x DMAs so the PE starts ASAP
            alloc_block(0)
            for k in range(KT):
                nc.sync.dma_start(wq_sb[:, k], wq_r[:, k])
                nc.sync.dma_start(T["xt", 0][:, k], xt_r[:, k, 0:SB])
            for k in range(KT):
                nc.sync.dma_start(wk_sb[:, k], wk_r[:, k])
            nc.sync.dma_start(wv_sb, wv_r)
            nc.sync.dma_start(wo_sb, wo.rearrange("(mt p) n -> p mt n", p=128))

            def stt(out_ap, e_ap, r_ap, tmp_ap):
                # out = min(e, 1) + r
                if stt_eng == "gpsimd":
                    # TensorScalarPtr isn't legal on Pool; use two TT ops
                    nc.gpsimd.tensor_tensor(tmp_ap, e_ap, onesb, ALU.min)
                    nc.gpsimd.tensor_add(out_ap, tmp_ap, r_ap)
                else:
                    nc.vector.scalar_tensor_tensor(
                        out=out_ap, in0=e_ap, scalar=1.0, in1=r_ap,
                        op0=ALU.min, op1=ALU.add,
                    )

            # ---- projection-phase steps for block blk ---------------------
            def stage_p_steps(blk, skip_dma=False):
                ssl = slice(blk * SB, (blk + 1) * SB)

                if not skip_dma:
                    def dma_step():
                        alloc_block(blk)
                        nc.sync.dma_start(T["xt", blk], xt_r[:, :, ssl])
                    yield dma_step

                for di in range(DT_TILES):
                    for pname, w_sb, bcol, dkey in (
                        ("q", wq_sb, 0, "qt"),
                        ("k", wk_sb, 4, "kt"),
                    ):
                        def qk_step(di=di, pname=pname, w_sb=w_sb,
                                    bcol=bcol, dkey=dkey):
                            dsl = slice(di * 128, (di + 1) * 128)
                            xt_t = T["xt", blk]
                            dst = T[dkey, blk]
                            ps = psA.tile([128, SB], F32, tag="proj",
                                          name=f"ps_{pname}{di}_{blk}")
                            for k in range(KT):
                                nc.tensor.matmul(
                                    ps, w_sb[:, k, dsl], xt_t[:, k],
                                    start=(k == 0), stop=(k == KT - 1),
                                )
                            bias_ap = cpk[:, 512 + bcol + di:513 + bcol + di]
                            # elu(u)+1 = min(exp(u),1) + max(u,0), u = ps+bias
                            e = evac.tile([128, SB], DT, tag="e",
                                          name=f"e_{pname}{di}_{blk}")
                            nc.scalar.activation(e, ps, AF.Exp, bias=bias_ap)
                            r = evac.tile([128, SB], DT, tag="r",
                                          name=f"r_{pname}{di}_{blk}")
                            nc.scalar.activation(r, ps, AF.Relu, bias=bias_ap)
                            tmp = evac.tile([128, SB], DT, tag="t",
                                            name=f"t_{pname}{di}_{blk}")
                            stt(dst[:, di], e, r, tmp)
                        yield qk_step

                # natural-layout V via direct (natural-out) projection;
                # bias folded in as a ones-row rank-1 matmul
                for st in range(NST):
                    def vnat_step(st=st):
                        xt_t = T["xt", blk]
                        vnat = T["vnat", blk]
                        stsl = slice(st * 128, (st + 1) * 128)
                        ps = psA.tile([128, MC], F32, tag="proj",
                                      name=f"ps_vn{st}_{blk}")
                        for k in range(KT):
                            nc.tensor.matmul(
                                ps, xt_t[:, k, stsl], wv_sb[:, k],
                                start=(k == 0), stop=False,
                                skip_group_check=True,
                            )
                        nc.tensor.matmul(
                            ps, ones32, bvrow_sb,
                            start=False, stop=True, skip_group_check=True,
                        )
                        nc.scalar.copy(
                            vnat.rearrange("p st (h e) -> p st h e", e=65)
                                [:, st, :, 0:64],
                            ps.rearrange("p (h e) -> p h e", e=64),
                        )
                    yield vnat_step

            # ---- attention slot for global chunk s ------------------------
            # head h: di = h//2, partitions pr = (h%2)*64..+64
            def pr(h):
                return slice((h % 2) * 64, (h % 2) * 64 + 64)

            def atm_off(h):
                # even heads in at_ev -> cols 0:512; odd in at_od -> 512:1024
                return (h % 2) * 512 + (h // 2) * 128

            def emit_slot(s, p_queue):
                blk, ccb = divmod(s, NST)
                csl = slice(ccb * 128, (ccb + 1) * 128)
                kt_t, qt_t = T["kt", blk], T["qt", blk]
                knat, vnat = T["knat", blk], T["vnat", blk]
                attn_t = T["attn", blk]

                # --- knat transposes for this chunk (packed, one bank) ----
                trk = psB.tile([128, 512], DT, tag="atq", name=f"trk_{s}")
                for di in range(DT_TILES):
                    nc.tensor.matmul(
                        trk[:, di * 128:(di + 1) * 128],
                        kt_t[:, di, csl], ident, is_transpose=True,
                        start=(di == 0), stop=(di == DT_TILES - 1),
                        skip_group_check=True,
                    )
                nc.vector.tensor_copy(knat[:, ccb], trk)

                # --- attn transposes for previous chunk -------------------
                if s > 0:
                    emit_trA(s - 1)

                # --- AT for 8 heads: evens -> at_ev bank, odds -> at_od ---
                at_ev = psB.tile([128, 512], F32, tag="atq", name=f"atev_{s}")
                at_od = psB.tile([128, 512], F32, tag="atq", name=f"atod_{s}")
                for idx, bank in ((0, at_ev), (1, at_od)):
                    for j in range(4):
                        h = 2 * j + idx
                        nc.tensor.matmul(
                            bank[:, j * 128:(j + 1) * 128],
                            kt_t[pr(h), h // 2, csl],
                            qt_t[pr(h), h // 2, csl],
                            start=(j == 0), stop=(j == 3),
                            skip_group_check=True,
                        )
                at_m = small.tile([128, 1024], DT, tag="atm", name=f"atm_{s}")
                nc.vector.tensor_mul(at_m[:, 0:512], at_ev, mask_ap)
                nc.vector.tensor_mul(at_m[:, 512:1024], at_od, mask_ap)

                # --- acc: inter (q @ [S|z]) then intra (AT_m @ [V|1]) -----
                # head h lives in bank (h<4 ? lo : hi) at cols
                # (h//2 % 2)*130 + (h%2)*65, so dens sit at 64::65 as before.
                acc_lo = psB.tile([128, 260], F32, tag="acc", name=f"accl_{s}")
                acc_hi = psB.tile([128, 260], F32, tag="acc", name=f"acch_{s}")

                def acc_t(h):
                    return acc_lo if h < 4 else acc_hi

                def acc_off(h):
                    return (h // 2 % 2) * 130 + (h % 2) * 65

                if s > 0:
                    # per-pair matmul: full-128 stationary against the
                    # block-diagonal state snapshot (uniform PE row group —
                    # row-split matmuls into one bank are illegal)
                    for hp in range(HPC // 2):
                        nc.tensor.matmul(
                            acc_t(2 * hp)[:, (hp % 2) * 130:(hp % 2) * 130 + 130],
                            qt_t[:, hp, csl],
                            ssb[:, hp],
                            start=(hp % 2 == 0), stop=False,
                            skip_group_check=True,
                        )

                # interleave some projection work for PE cover
                for _ in range(2):
                    if p_queue:
                        p_queue.pop(0)()

                for h in range(HPC):
                    a = h % 4
                    o = atm_off(h)
                    nc.tensor.matmul(
                        acc_t(h)[:, acc_off(h):acc_off(h) + 65],
                        at_m[:, o:o + 128],
                        vnat[:, ccb, h * 65:(h + 1) * 65],
                        start=(s == 0 and a == 0), stop=(a == 3),
                        skip_group_check=True,
                    )

                # --- state update: [S|z] += K_c^T [V|1], psum accumulate --
                if s < NCH - 1:
                    for h in range(HPC):
                        nc.tensor.matmul(
                            state_ps[pr(h), (h // 2) * 128:(h // 2) * 128 + 65],
                            knat[:, ccb, h * 64:(h + 1) * 64],
                            vnat[:, ccb, h * 65:(h + 1) * 65],
                            start=False,
                            stop=(s == NCH - 2 and h == HPC - 1),
                            skip_group_check=True,
                            tile_position=(0, (h % 2) * 64),
                        )
                    sview = state_ps.rearrange("p (a c) -> p a c", c=128)
                    nc.vector.tensor_copy(
                        ssb[0:64, :, 0:65], sview[0:64, :, 0:65])
                    nc.vector.tensor_copy(
                        ssb[64:128, :, 65:130], sview[64:128, :, 0:65])

                for _ in range(2):
                    if p_queue:
                        p_queue.pop(0)()

                # --- normalize: recip on DVE, per-head scale on Scalar ----
                rec = small.tile([128, 8], F32, tag="rec", name=f"rec_{s}")
                nc.vector.reciprocal(rec[:, 0:4], acc_lo[:, 64:260:65])
                nc.vector.reciprocal(rec[:, 4:8], acc_hi[:, 64:260:65])
                for h in range(HPC):
                    nc.scalar.mul(
                        attn_t[:, ccb, h * 64:(h + 1) * 64],
                        acc_t(h)[:, acc_off(h):acc_off(h) + 64],
                        rec[:, h:h + 1],
                    )

                # --- output projection for previous chunk -----------------
                if s > 0:
                    emit_oproj(s - 1)

            def emit_trA(s):
                pblk, pccb = divmod(s, NST)
                pcsl = slice(pccb * 128, (pccb + 1) * 128)
                trA = psB.tile([128, 512], DT, tag="atq", name=f"trA_{s}")
                for p in range(DT_TILES):
                    nc.tensor.matmul(
                        trA[:, p * 128:(p + 1) * 128],
                        T["attn", pblk][:, pccb, p * 128:(p + 1) * 128],
                        ident, is_transpose=True,
                        start=(p == 0), stop=(p == DT_TILES - 1),
                        skip_group_check=True,
                    )
                nc.vector.tensor_copy(
                    T["attnT", pblk][:, :, pcsl],
                    trA.rearrange("p (a b) -> p a b", a=DT_TILES),
                )

            def emit_oproj(s):
                blk, ccb = divmod(s, NST)
                csl = slice(ccb * 128, (ccb + 1) * 128)
                attnT = T["attnT", blk]
                ob = ob_pool.tile([128, D], BF16, tag="ob", name=f"ob_{s}")
                for nb in range(2):
                    nsl = slice(nb * 512, (nb + 1) * 512)
                    ops = psA.tile([128, 512], F32, tag="proj",
                                   name=f"ops_{s}_{nb}")
                    for p in range(DT_TILES):
                        nc.tensor.matmul(
                            ops, attnT[:, p, csl], wo_sb[:, p, nsl],
                            start=(p == 0), stop=(p == DT_TILES - 1),
                        )
                    nc.vector.tensor_copy(ob[:, nsl], ops)
                nc.sync.dma_start(out_r[s], ob)

            # ---- emission: P(0), then slots with P(blk+1) interleave ------
            for step in stage_p_steps(0, skip_dma=True):
                step()
            p_queue = []
            for s in range(NCH):
                blk, ccb = divmod(s, NST)
                if ccb == 0 and blk + 1 < NBLK:
                    p_queue.extend(stage_p_steps(blk + 1))
                emit_slot(s, p_queue)
                while p_queue and ccb == NST - 1:
                    p_queue.pop(0)()
            emit_trA(NCH - 1)
            emit_oproj(NCH - 1)

    nc.compile()
    return nc


def _prep_inputs(x, Wq, bq, Wk, bk, Wv, bv, Wo, bo, np_dt):
    f32 = np.float32
    tri = np.triu(np.ones((128, 128), f32))  # mask[j,i] = 1 iff j <= i
    mask4 = np.tile(tri, (1, 4))             # 4 heads per mask op
    ident = np.eye(128, dtype=np_dt)
    in_maps = []
    for c in range(8):
        b, hh = divmod(c, 2)
        cols = slice(hh * MC, (hh + 1) * MC)
        bqkv = np.concatenate(
            [np.asarray(v[cols], f32).reshape(4, 128).T for v in (bq, bk, bv)],
            axis=1,
        ).astype(f32)
        cpack = np.concatenate([mask4, bqkv], axis=1).astype(f32)
        in_maps.append({
            "xt": np.ascontiguousarray(np.asarray(x[b], f32).T).astype(np_dt),
            "wq": np.ascontiguousarray(np.asarray(Wq, f32)[:, cols]).astype(np_dt),
            "wk": np.ascontiguousarray(np.asarray(Wk, f32)[:, cols]).astype(np_dt),
            "wv": np.ascontiguousarray(np.asarray(Wv, f32)[:, cols]).astype(np_dt),
            "wo": np.ascontiguousarray(np.asarray(Wo, f32)[cols, :]).astype(np_dt),
            "cpack": np.ascontiguousarray(cpack),
            "bvrow": np.ascontiguousarray(np.concatenate(
                [np.asarray(bv, f32)[cols][None, :],
                 np.zeros((31, MC), f32)], axis=0)).astype(np_dt),
            "idn": ident,
        })
    return in_maps


def run(inputs, trace=False):
    """Run the kernel; returns (full_output, BassKernelResults)."""
    from concourse.bass_utils import run_bass_kernel_spmd

    dt_name = DT_NAME
    if dt_name not in _built:
        _built[dt_name] = _build(dt_name)
    nc = _built[dt_name]

    x = np.asarray(inputs["x"], np.float32)
    bo = np.asarray(inputs["bo"], np.float32)
    in_maps = _prep_inputs(
        x, inputs["Wq"], inputs["bq"], inputs["Wk"], inputs["bk"],
        inputs["Wv"], inputs["bv"], inputs["Wo"], bo, _np_dt(dt_name),
    )
    res = run_bass_kernel_spmd(
        nc, in_maps, core_ids=list(range(8)), trace=trace,
        trace_cores=list(range(8)) if trace else None,
    )
    outs = [np.asarray(r["out"], np.float32) for r in res.results]
    full = np.empty((B, S, D), np.float32)
    for b in range(B):
        full[b] = outs[2 * b] + outs[2 * b + 1] + bo[None, :]
    return full, res


def kernel(**inputs):
    full, _ = run(inputs, trace=False)
    return full


# revision 35
# speedup vs baseline: 1.3778x; 1.0286x over previous
"""Trainium2 Bass kernel for MultiHeadLinearAttention.

Problem: B=4, S=2048, D=1024, H=16 heads of hd=64.
  q,k,v = x@W + b ; q,k = elu(q|k)+1
  recurrent scan: s += k_t v_t^T ; z += k_t ; out_t = (q_t s)/(q_t z + 1e-6)
  y = out @ Wo + bo

Strategy (8 NeuronCores):
  core c -> batch b = c//2, heads hh = (c%2)*8 .. +8  (column-sliced Wq/Wk/Wv,
  row-sliced Wo; the two cores of a batch produce partial output-projection
  sums that the host adds together, plus bo).

  Linear attention in chunked form, chunk C=128, software-pipelined at chunk
  granularity:
    AT[j,i] = sum_d k[j,d] q[i,d]   (masked j<=i, 4 heads per PSUM bank,
                                     ONE mask-multiply per bank)
    acc[i, h*65:+65] = [ q_i @ [S|z]_pre  +  sum_j AT_m[j,i] [v_j|1] ]
    out_i = acc[..0:64] * recip(acc[..64])    (recip on DVE, scale on Scalar)
    [S|z] += K_c^T [V_c|1]   accumulated IN PSUM across all chunks; one
                             f32->bf16 copy per chunk materializes the
                             state snapshot used by the next chunk's inter.
  elu(x)+1 == min(exp(x),1) + max(x,0)  (2 scalar acts + 1 gpsimd STT).
  V-bias is a rank-1 ones-row matmul folded into the V projection.
  Output projection consumes PE-transposed attn tiles; y is written bf16.
"""

import os
import sys

import numpy as np
import ml_dtypes

sys.path.insert(0, "/opt/trn_rl_repo")

B, S, D = 4, 2048, 1024
H, HD = 16, 64
HPC = 8           # heads per core
MC = HPC * HD     # 512 head-dims per core
C = 128           # attention chunk
SB = 512          # s-block (projection granularity)
NBLK = S // SB    # 4
NST = SB // C     # chunks per block
NCH = S // C      # 16 global chunks

# matmul operand dtype: "bfloat16" | "float32r" | "float32"
DT_NAME = os.environ.get("BASS_LINATTN_DT", "bfloat16")
STT_ENGINE = os.environ.get("BASS_LINATTN_STT", "vector")

_built = {}


def _np_dt(name):
    return {"bfloat16": ml_dtypes.bfloat16,
            "float32r": np.float32,
            "float32": np.float32}[name]


def _build(dt_name):
    import concourse.bass as bass
    import concourse.mybir as mybir
    from concourse import bacc
    from concourse.tile import TileContext

    DT = getattr(mybir.dt, dt_name)
    BF16 = mybir.dt.bfloat16
    F32 = mybir.dt.float32
    AF = mybir.ActivationFunctionType
    ALU = mybir.AluOpType

    nc = bacc.Bacc("TRN2", target_bir_lowering=False, debug=False)

    xt = nc.dram_tensor("xt", (D, S), DT, kind="ExternalInput")
    wq = nc.dram_tensor("wq", (D, MC), DT, kind="ExternalInput")
    wk = nc.dram_tensor("wk", (D, MC), DT, kind="ExternalInput")
    wv = nc.dram_tensor("wv", (D, MC), DT, kind="ExternalInput")
    wo = nc.dram_tensor("wo", (MC, D), DT, kind="ExternalInput")
    # cpack = [ mask (512 cols: 4x triu) | biases (12 cols: q di, k di, v di) ]
    cpack = nc.dram_tensor("cpack", (128, 524), F32, kind="ExternalInput")
    bvrow = nc.dram_tensor("bvrow", (32, MC), DT, kind="ExternalInput")
    idn = nc.dram_tensor("idn", (128, 128), DT, kind="ExternalInput")
    out = nc.dram_tensor("out", (S, D), BF16, kind="ExternalOutput")

    KT = D // 128          # 8 k-tiles of the contraction dim
    DT_TILES = MC // 128   # 4 tiles of per-core head dims

    stt_eng = "gpsimd" if STT_ENGINE == "gpsimd" else "vector"

    with TileContext(nc) as tc:
        with (
            tc.tile_pool(name="consts", bufs=1) as consts,
            tc.tile_pool(name="xt_pool", bufs=2) as xt_pool,
            tc.tile_pool(name="qkvt", bufs=2) as qkvt,
            tc.tile_pool(name="nat", bufs=2) as nat,
            tc.tile_pool(name="attn_pool", bufs=2) as attn_pool,
            tc.tile_pool(name="attnT_pool", bufs=2) as attnT_pool,
            tc.tile_pool(name="small", bufs=2) as small,
            tc.tile_pool(name="evac", bufs=2) as evac,
            tc.tile_pool(name="ob_pool", bufs=2) as ob_pool,
            tc.tile_pool(name="psA", bufs=3, space="PSUM") as psA,
            tc.tile_pool(name="psB", bufs=2, space="PSUM") as psB,
        ):
            # ---- constant tiles -------------------------------------------
            cpk = consts.tile([128, 524], F32)
            ident = consts.tile([128, 128], DT)
            bvrow_sb = consts.tile([32, MC], DT)
            ones32 = consts.tile([32, 128], DT)
            onesb = consts.tile([128, SB], DT)
            wq_sb = consts.tile([128, KT, MC], DT)
            wk_sb = consts.tile([128, KT, MC], DT)
            wv_sb = consts.tile([128, KT, MC], DT)
            wo_sb = consts.tile([128, DT_TILES, D], DT)

            mask_ap = cpk[:, 0:512]

            # ---- persistent [S|z] state: f32 accumulation in PSUM ---------
            # full-bank stride [128, 512]; pair hp's [S|z] lives at cols
            # hp*128 .. hp*128+65 (head 2hp on partitions 0:64, 2hp+1 on 64:128)
            state_ps = psB.tile([128, 4 * 128], F32, tag="state", bufs=1)

            # ---- DMAs: startup-critical first; weights go out on the
            # Activation HWDGE queue so issue overlaps with x on Sync -------
            nc.scalar.dma_start(cpk, cpack[:, :])
            nc.sync.dma_start(ident, idn[:, :])
            nc.scalar.dma_start(bvrow_sb, bvrow[:, :])
            nc.gpsimd.memset(ones32, 0.0)
            nc.gpsimd.memset(ones32[0:1], 1.0)
            nc.gpsimd.memset(onesb, 1.0)
            # zero the state bank; every state update accumulates with
            # start=False (overwrite-of-0 and add-to-0 are equivalent)
            nc.vector.memset(state_ps, 0.0)
            # block-diagonal bf16 snapshot of [S|z] per pair: rows 0:64 hold
            # head 2hp at cols 0:65, rows 64:128 hold head 2hp+1 at 65:130.
            # Off-diagonal zeros are set once and never rewritten.
            ssb = consts.tile([128, HPC // 2, 130], DT)
            nc.vector.memset(ssb, 0.0)

            xt_r = xt.rearrange("(kt p) s -> p kt s", p=128)
            wq_r = wq.rearrange("(kt p) n -> p kt n", p=128)
            wk_r = wk.rearrange("(kt p) n -> p kt n", p=128)
            wv_r = wv.rearrange("(kt p) n -> p kt n", p=128)
            out_r = out.rearrange("(st p) n -> st p n", p=128)

            T = {}

            def alloc_block(blk):
                T["xt", blk] = xt_pool.tile([128, KT, SB], DT, tag="xt",
                                            name=f"xt_{blk}")
                T["qt", blk] = qkvt.tile([128, DT_TILES, SB], DT, tag="qt",
                                         name=f"qt_{blk}")
                T["kt", blk] = qkvt.tile([128, DT_TILES, SB], DT, tag="kt",
                                         name=f"kt_{blk}")
                T["knat", blk] = nat.tile([128, NST, MC], DT, tag="knat",
                                          name=f"knat_{blk}")
                vnat = nat.tile([128, NST, HPC * 65], DT, tag="vnat",
                                name=f"vnat_{blk}")
                T["vnat", blk] = vnat
                nc.gpsimd.memset(
                    vnat.rearrange("p st (h e) -> p st h e", e=65)
                        [:, :, :, 64:65], 1.0
                )
                T["attn", blk] = attn_pool.tile([128, NST, MC], DT, tag="attn",
                                                name=f"attn_{blk}")
                T["attnT", blk] = attnT_pool.tile([128, DT_TILES, SB], DT,
                                                  tag="attnT",
                                                  name=f"attnT_{blk}")

            # block 0: per-ktile weight/x DMAs so the PE starts ASAP
            alloc_block(0)
            for k in range(KT):
                nc.scalar.dma_start(wq_sb[:, k], wq_r[:, k])
                nc.sync.dma_start(T["xt", 0][:, k], xt_r[:, k, 0:SB])
            for k in range(KT):
                nc.scalar.dma_start(wk_sb[:, k], wk_r[:, k])
            nc.scalar.dma_start(wv_sb, wv_r)
            nc.scalar.dma_start(wo_sb, wo.rearrange("(mt p) n -> p mt n", p=128))

            def stt(out_ap, e_ap, r_ap, tmp_ap):
                # out = min(e, 1) + r
                if stt_eng == "gpsimd":
                    # TensorScalarPtr isn't legal on Pool; use two TT ops
                    nc.gpsimd.tensor_tensor(tmp_ap, e_ap, onesb, ALU.min)
                    nc.gpsimd.tensor_add(out_ap, tmp_ap, r_ap)
                else:
                    nc.vector.scalar_tensor_tensor(
                        out=out_ap, in0=e_ap, scalar=1.0, in1=r_ap,
                        op0=ALU.min, op1=ALU.add,
                    )

            # ---- projection-phase steps for block blk ---------------------
            # returns (main_steps, vnat_steps); vnat(blk, st) is only needed
            # by slot blk*NST+st, so those are deferred as late PE filler
            def stage_p_steps(blk, skip_dma=False):
                main, vnats = [], []
                ssl = slice(blk * SB, (blk + 1) * SB)

                if not skip_dma:
                    def dma_step():
                        alloc_block(blk)
                        nc.sync.dma_start(T["xt", blk], xt_r[:, :, ssl])
                    main.append(dma_step)

                for di in range(DT_TILES):
                    for pname, w_sb, bcol, dkey in (
                        ("q", wq_sb, 0, "qt"),
                        ("k", wk_sb, 4, "kt"),
                    ):
                        def qk_step(di=di, pname=pname, w_sb=w_sb,
                                    bcol=bcol, dkey=dkey):
                            dsl = slice(di * 128, (di + 1) * 128)
                            xt_t = T["xt", blk]
                            dst = T[dkey, blk]
                            ps = psA.tile([128, SB], F32, tag="proj",
                                          name=f"ps_{pname}{di}_{blk}")
                            for k in range(KT):
                                nc.tensor.matmul(
                                    ps, w_sb[:, k, dsl], xt_t[:, k],
                                    start=(k == 0), stop=(k == KT - 1),
                                )
                            bias_ap = cpk[:, 512 + bcol + di:513 + bcol + di]
                            # elu(u)+1 = min(exp(u),1) + max(u,0), u = ps+bias
                            e = evac.tile([128, SB], DT, tag="e",
                                          name=f"e_{pname}{di}_{blk}")
                            nc.scalar.activation(e, ps, AF.Exp, bias=bias_ap)
                            r = evac.tile([128, SB], DT, tag="r",
                                          name=f"r_{pname}{di}_{blk}")
                            nc.scalar.activation(r, ps, AF.Relu, bias=bias_ap)
                            tmp = evac.tile([128, SB], DT, tag="t",
                                            name=f"t_{pname}{di}_{blk}")
                            stt(dst[:, di], e, r, tmp)
                        main.append(qk_step)

                # natural-layout V via direct (natural-out) projection;
                # bias folded in as a ones-row rank-1 matmul
                for st in range(NST):
                    def vnat_step(st=st):
                        xt_t = T["xt", blk]
                        vnat = T["vnat", blk]
                        stsl = slice(st * 128, (st + 1) * 128)
                        ps = psA.tile([128, MC], F32, tag="proj",
                                      name=f"ps_vn{st}_{blk}")
                        for k in range(KT):
                            nc.tensor.matmul(
                                ps, xt_t[:, k, stsl], wv_sb[:, k],
                                start=(k == 0), stop=False,
                                skip_group_check=True,
                            )
                        nc.tensor.matmul(
                            ps, ones32, bvrow_sb,
                            start=False, stop=True, skip_group_check=True,
                        )
                        nc.scalar.copy(
                            vnat.rearrange("p st (h e) -> p st h e", e=65)
                                [:, st, :, 0:64],
                            ps.rearrange("p (h e) -> p h e", e=64),
                        )
                    vnats.append(vnat_step)
                return main, vnats

            # ---- attention slot for global chunk s ------------------------
            # head h: di = h//2, partitions pr = (h%2)*64..+64
            def pr(h):
                return slice((h % 2) * 64, (h % 2) * 64 + 64)

            def atm_off(h):
                # even heads in at_ev -> cols 0:512; odd in at_od -> 512:1024
                return (h % 2) * 512 + (h // 2) * 128

            q_early = []   # dma + qk steps of the next block
            q_late = []    # (due_slot, vnat_step)

            def pump_late(s):
                while q_late and q_late[0][0] <= s:
                    q_late.pop(0)[1]()

            def emit_slot(s):
                pump_late(s)
                blk, ccb = divmod(s, NST)
                csl = slice(ccb * 128, (ccb + 1) * 128)
                kt_t, qt_t = T["kt", blk], T["qt", blk]
                knat, vnat = T["knat", blk], T["vnat", blk]
                attn_t = T["attn", blk]

                # --- knat transposes for this chunk (packed, one bank) ----
                trk = psB.tile([128, 512], DT, tag="atq", name=f"trk_{s}")
                for di in range(DT_TILES):
                    nc.tensor.matmul(
                        trk[:, di * 128:(di + 1) * 128],
                        kt_t[:, di, csl], ident, is_transpose=True,
                        start=(di == 0), stop=(di == DT_TILES - 1),
                        skip_group_check=True,
                    )
                nc.vector.tensor_copy(knat[:, ccb], trk)

                # --- attn transposes for previous chunk -------------------
                if s > 0:
                    emit_trA(s - 1)

                # --- AT for 8 heads: evens -> at_ev bank, odds -> at_od ---
                at_ev = psB.tile([128, 512], F32, tag="atq", name=f"atev_{s}")
                at_od = psB.tile([128, 512], F32, tag="atq", name=f"atod_{s}")
                for idx, bank in ((0, at_ev), (1, at_od)):
                    for j in range(4):
                        h = 2 * j + idx
                        nc.tensor.matmul(
                            bank[:, j * 128:(j + 1) * 128],
                            kt_t[pr(h), h // 2, csl],
                            qt_t[pr(h), h // 2, csl],
                            start=(j == 0), stop=(j == 3),
                            skip_group_check=True,
                        )
                at_m = small.tile([128, 1024], DT, tag="atm", name=f"atm_{s}")
                nc.vector.tensor_mul(at_m[:, 0:512], at_ev, mask_ap)
                nc.vector.tensor_mul(at_m[:, 512:1024], at_od, mask_ap)

                # --- acc: inter (q @ [S|z]) then intra (AT_m @ [V|1]) -----
                # head h lives in bank (h<4 ? lo : hi) at cols
                # (h//2 % 2)*130 + (h%2)*65, so dens sit at 64::65 as before.
                acc_lo = psB.tile([128, 260], F32, tag="acc", name=f"accl_{s}")
                acc_hi = psB.tile([128, 260], F32, tag="acc", name=f"acch_{s}")

                def acc_t(h):
                    return acc_lo if h < 4 else acc_hi

                def acc_off(h):
                    return (h // 2 % 2) * 130 + (h % 2) * 65

                if s > 0:
                    # per-pair matmul: full-128 stationary against the
                    # block-diagonal state snapshot (uniform PE row group —
                    # row-split matmuls into one bank are illegal)
                    for hp in range(HPC // 2):
                        nc.tensor.matmul(
                            acc_t(2 * hp)[:, (hp % 2) * 130:(hp % 2) * 130 + 130],
                            qt_t[:, hp, csl],
                            ssb[:, hp],
                            start=(hp % 2 == 0), stop=False,
                            skip_group_check=True,
                        )

                # interleave some projection work for PE cover
                for _ in range(2):
                    if q_early:
                        q_early.pop(0)()

                for h in range(HPC):
                    a = h % 4
                    o = atm_off(h)
                    nc.tensor.matmul(
                        acc_t(h)[:, acc_off(h):acc_off(h) + 65],
                        at_m[:, o:o + 128],
                        vnat[:, ccb, h * 65:(h + 1) * 65],
                        start=(s == 0 and a == 0), stop=(a == 3),
                        skip_group_check=True,
                    )

                # --- state update: [S|z] += K_c^T [V|1], psum accumulate --
                if s < NCH - 1:
                    for h in range(HPC):
                        nc.tensor.matmul(
                            state_ps[pr(h), (h // 2) * 128:(h // 2) * 128 + 65],
                            knat[:, ccb, h * 64:(h + 1) * 64],
                            vnat[:, ccb, h * 65:(h + 1) * 65],
                            start=False,
                            stop=(s == NCH - 2 and h == HPC - 1),
                            skip_group_check=True,
                            tile_position=(0, (h % 2) * 64),
                        )
                    sview = state_ps.rearrange("p (a c) -> p a c", c=128)
                    nc.vector.tensor_copy(
                        ssb[0:64, :, 0:65], sview[0:64, :, 0:65])
                    nc.vector.tensor_copy(
                        ssb[64:128, :, 65:130], sview[64:128, :, 0:65])

                for _ in range(2):
                    if q_early:
                        q_early.pop(0)()
                if q_late and q_late[0][0] <= s + 1:
                    q_late.pop(0)[1]()

                # --- normalize: recip on DVE, per-head scale on Scalar ----
                rec = small.tile([128, 8], F32, tag="rec", name=f"rec_{s}")
                nc.vector.reciprocal(rec[:, 0:4], acc_lo[:, 64:260:65])
                nc.vector.reciprocal(rec[:, 4:8], acc_hi[:, 64:260:65])
                for h in range(HPC):
                    nc.scalar.mul(
                        attn_t[:, ccb, h * 64:(h + 1) * 64],
                        acc_t(h)[:, acc_off(h):acc_off(h) + 64],
                        rec[:, h:h + 1],
                    )

                # --- output projection for previous chunk -----------------
                if s > 0:
                    emit_oproj(s - 1)

            def emit_trA(s):
                pblk, pccb = divmod(s, NST)
                pcsl = slice(pccb * 128, (pccb + 1) * 128)
                trA = psB.tile([128, 512], DT, tag="atq", name=f"trA_{s}")
                for p in range(DT_TILES):
                    nc.tensor.matmul(
                        trA[:, p * 128:(p + 1) * 128],
                        T["attn", pblk][:, pccb, p * 128:(p + 1) * 128],
                        ident, is_transpose=True,
                        start=(p == 0), stop=(p == DT_TILES - 1),
                        skip_group_check=True,
                    )
                nc.vector.tensor_copy(
                    T["attnT", pblk][:, :, pcsl],
                    trA.rearrange("p (a b) -> p a b", a=DT_TILES),
                )

            def emit_oproj(s):
                blk, ccb = divmod(s, NST)
                csl = slice(ccb * 128, (ccb + 1) * 128)
                attnT = T["attnT", blk]
                ob = ob_pool.tile([128, D], BF16, tag="ob", name=f"ob_{s}")
                for nb in range(2):
                    nsl = slice(nb * 512, (nb + 1) * 512)
                    ops = psA.tile([128, 512], F32, tag="proj",
                                   name=f"ops_{s}_{nb}")
                    for p in range(DT_TILES):
                        nc.tensor.matmul(
                            ops, attnT[:, p, csl], wo_sb[:, p, nsl],
                            start=(p == 0), stop=(p == DT_TILES - 1),
                        )
                    nc.vector.tensor_copy(ob[:, nsl], ops)
                nc.sync.dma_start(out_r[s], ob)

            # ---- emission: P(0), then slots with P(blk+1) interleave ------
            main0, vnat0 = stage_p_steps(0, skip_dma=True)
            for step in main0:
                step()
            vnat0[0]()
            q_late.extend((st, step) for st, step in enumerate(vnat0) if st > 0)
            for s in range(NCH):
                blk, ccb = divmod(s, NST)
                if ccb == 0 and blk + 1 < NBLK:
                    m, v = stage_p_steps(blk + 1)
                    q_early.extend(m)
                    q_late.extend(
                        ((blk + 1) * NST + st, step) for st, step in enumerate(v)
                    )
                emit_slot(s)
                while q_early and ccb == NST - 1:
                    q_early.pop(0)()
            emit_trA(NCH - 1)
            emit_oproj(NCH - 1)

    nc.compile()
    return nc


def _prep_inputs(x, Wq, bq, Wk, bk, Wv, bv, Wo, bo, np_dt):
    f32 = np.float32
    tri = np.triu(np.ones((128, 128), f32))  # mask[j,i] = 1 iff j <= i
    mask4 = np.tile(tri, (1, 4))             # 4 heads per mask op
    ident = np.eye(128, dtype=np_dt)
    in_maps = []
    for c in range(8):
        b, hh = divmod(c, 2)
        cols = slice(hh * MC, (hh + 1) * MC)
        bqkv = np.concatenate(
            [np.asarray(v[cols], f32).reshape(4, 128).T for v in (bq, bk, bv)],
            axis=1,
        ).astype(f32)
        cpack = np.concatenate([mask4, bqkv], axis=1).astype(f32)
        in_maps.append({
            "xt": np.ascontiguousarray(np.asarray(x[b], f32).T).astype(np_dt),
            "wq": np.ascontiguousarray(np.asarray(Wq, f32)[:, cols]).astype(np_dt),
            "wk": np.ascontiguousarray(np.asarray(Wk, f32)[:, cols]).astype(np_dt),
            "wv": np.ascontiguousarray(np.asarray(Wv, f32)[:, cols]).astype(np_dt),
            "wo": np.ascontiguousarray(np.asarray(Wo, f32)[cols, :]).astype(np_dt),
            "cpack": np.ascontiguousarray(cpack),
            "bvrow": np.ascontiguousarray(np.concatenate(
                [np.asarray(bv, f32)[cols][None, :],
                 np.zeros((31, MC), f32)], axis=0)).astype(np_dt),
            "idn": ident,
        })
    return in_maps


def run(inputs, trace=False):
    """Run the kernel; returns (full_output, BassKernelResults)."""
    from concourse.bass_utils import run_bass_kernel_spmd

    dt_name = DT_NAME
    if dt_name not in _built:
        _built[dt_name] = _build(dt_name)
    nc = _built[dt_name]

    x = np.asarray(inputs["x"], np.float32)
    bo = np.asarray(inputs["bo"], np.float32)
    in_maps = _prep_inputs(
        x, inputs["Wq"], inputs["bq"], inputs["Wk"], inputs["bk"],
        inputs["Wv"], inputs["bv"], inputs["Wo"], bo, _np_dt(dt_name),
    )
    res = run_bass_kernel_spmd(
        nc, in_maps, core_ids=list(range(8)), trace=trace,
        trace_cores=list(range(8)) if trace else None,
    )
    outs = [np.asarray(r["out"], np.float32) for r in res.results]
    full = np.empty((B, S, D), np.float32)
    for b in range(B):
        full[b] = outs[2 * b] + outs[2 * b + 1] + bo[None, :]
    return full, res


def kernel(**inputs):
    full, _ = run(inputs, trace=False)
    return full
